# revision 46
# baseline (speedup 1.0000x reference)
"""CapsuleLayer (dynamic routing, 3 iterations) on 8 Trainium2 NeuronCores.

Zero-collective design. A collective-based kernel spends ~60us waiting for
the ncfw/TOPSP firmware to boot plus ~10us per collective; with ~25us of
real math that dominates. This kernel eliminates every collective:

  - The routing statistics (b_ij += mean over 256 batch samples of u_hat.v)
    tolerate large per-element noise (it averages out ~16x in the batch
    mean), so each core REPLICATES the full-batch routing (iterations 1-2)
    in fp8 instead of sharding it (measured ~3e-3 final rel err vs the 2e-2
    gate, identical to bf16 routing).
  - Iteration 3 (output-determining) runs in bf16 with each core producing
    only its 32-row batch shard of v_3; the host concatenates. No
    ReduceScatter, no AllGather, no warm-up, no ncfw boot.

Layout: rows j = (i,k), 9216 rows = 72 chunks of 128. All DRAM inputs are
host-packed partition-major so every DMA is contiguous. The (n,o) capsule
columns are stored (o,n) — n innermost — so every broadcast-by-n multiply
(mc = c*wl, v = fac*s) has a step-1 innermost AP on both sources and hits
the DVE 2x_1P packed mode; the o-reduction tree becomes contiguous
block-halving adds.

DMA: only sync/scalar/gpsimd can issue (hwdge + sw-dge). Issue cost is
~620ns per dma_start regardless of size, so wl8+xt8 are host-packed into
ONE tensor (in8, per-chunk [wl8|xt8]) -> 8 slab issues spread over the 3
queues, chunk-ordered so the s1 matmul streams behind the DMA wave.

Engine facts this schedule is built on (measured on this hardware):
fp8 DoubleRow matmul ~135ns pitch (256-deep, 160 free); DVE 0.54 ns/col
only when every operand is 2-byte, packed, SBUF (broadcast-over-innermost
or PSUM reads drop it to 1.07-1.37; a concurrently streaming PE degrades
DVE SBUF access up to ~2.5x); GpSimd 1.95-2.1; Scalar copy 1.2-1.4.
  s/Q matmuls   fp8 DoubleRow; rhs (wl/mc) in [p,(o,n),c] layout.
  mc = c o wl   DVE packed multiplies (c broadcast over o sits OUTSIDE the
                innermost n dim); iter-1 is fp8 (1x regardless).
  Q egress      Scalar copies 3 of 4 PSUM banks to bf16 (DVE direct-
                multiplies the 4th); DVE packed-multiplies p = wlb o Q.
  pr = sum_o p  contiguous halving adds 160->80->40->20->10, level 1
                per-run on DVE (pipelined under the next Q groups).
  uv = F.T @ pr PE; F = kron(I16, ones8x8)/(B*SV) = 2^-12 exact in bf16
                (sums k inside i-groups, replicates back, folds all scales)
  squash        scalar ACT Square + Sqrt(scale=fs^2) + DVE reduce/recip;
                fs=SV folded exactly (pow2) into the Sqrt scale.
Scales: wl8 = 16*0.03*W keeps fp8 normals; v8 = 16*v; x unscaled.
"""
import sys

if "/opt/trn_rl_repo" not in sys.path:
    sys.path.insert(0, "/opt/trn_rl_repo")

import numpy as np

N_CORES = 8
B, IN_SIZE, I_TOT = 256, 8, 1152
N_NODE, O_SZ = 10, 16
NO = N_NODE * O_SZ          # 160
J = I_TOT * IN_SIZE         # 9216 rows (i,k)
NCH = J // 128              # 72 chunks
NG = NCH // 2               # 36 DoubleRow chunk-pairs
NH = NCH // 2               # 36 chunks per b_update half
B_SH = B // N_CORES         # 32 batch rows per core
SW = 16.0                   # wl fp8 scale
SV = 16.0                   # v fp8 scale
CW = NO + B                 # 416 packed cols per chunk in in8
RSQRT_MAGIC = 0x5F3759DF

_CACHE = {}
_DEBUG = False              # adds stage-dump outputs (debugging only)


def _build_program():
    import concourse.bacc as bacc
    import concourse.tile as tile
    import concourse.mybir as mybir

    f32 = mybir.dt.float32
    bf16 = mybir.dt.bfloat16
    f8 = mybir.dt.float8e4
    i32 = mybir.dt.int32
    AF = mybir.ActivationFunctionType
    ALU = mybir.AluOpType
    AX = mybir.AxisListType
    PM = mybir.MatmulPerfMode.DoubleRow

    nc = bacc.Bacc("TRN2", target_bir_lowering=False, debug=False,
                   enable_asserts=True, num_devices=N_CORES)

    in8_d = nc.dram_tensor("in8", [128, NCH * CW], f8,
                           kind="ExternalInput").ap()
    xik8_d = nc.dram_tensor("xik8", [128, 2 * J], f8,
                            kind="ExternalInput").ap()
    wlb_d = nc.dram_tensor("wlb", [128, NCH * NO], bf16,
                           kind="ExternalInput").ap()
    xts_d = nc.dram_tensor("xts", [128, NCH * B_SH], bf16,
                           kind="ExternalInput").ap()
    f_d = nc.dram_tensor("fmat", [128, 128], bf16, kind="ExternalInput").ap()
    y_d = nc.dram_tensor("y", [B_SH, NO], f32, kind="ExternalOutput").ap()
    dbg = {}
    if _DEBUG:
        for nm, sh, dt in [("d_s1", [128, 2 * NO], f32),
                           ("d_v1", [128, 2 * NO], f32),
                           ("d_b1", [128, NCH * N_NODE], f32),
                           ("d_c1", [128, NCH * N_NODE], f32),
                           ("d_q1", [128, NO], f32),
                           ("d_p1", [128, NO], f32),
                           ("d_pr1", [128, N_NODE], f32),
                           ("d_s2", [128, 2 * NO], f32),
                           ("d_b2", [128, NCH * N_NODE], f32),
                           ("d_s3", [B_SH, NO], f32)]:
            dbg[nm] = nc.dram_tensor(nm, sh, dt,
                                     kind="ExternalOutput").ap()

    with tile.TileContext(nc) as tc:
        with tc.tile_pool(name="persist", bufs=1) as pp, \
             tc.tile_pool(name="work", bufs=1) as wp, \
             tc.tile_pool(name="half", bufs=1) as hp, \
             tc.tile_pool(name="ps_s", bufs=1, space="PSUM") as ps_s, \
             tc.tile_pool(name="ps_q", bufs=4, space="PSUM") as ps_q, \
             tc.tile_pool(name="ps_f", bufs=1, space="PSUM") as ps_f:

            in8_sb = pp.tile([128, NCH, CW], f8, name="in8_sb",
                             tag="in8_sb")
            xik8_sb = pp.tile([128, 2, J], f8, name="xik8_sb", tag="xik8_sb")
            wlb_sb = pp.tile([128, NCH, NO], bf16, name="wlb_sb",
                             tag="wlb_sb")
            xts_sb = pp.tile([128, NCH, B_SH], bf16, name="xts_sb",
                             tag="xts_sb")
            f_sb = pp.tile([128, 128], bf16, name="f_sb", tag="f_sb")
            b_sb = pp.tile([128, NCH, N_NODE], f32, name="b_sb", tag="b_sb")

            wl8_sb = in8_sb[:, :, 0:NO]
            xt8_sb = in8_sb[:, :, NO:CW]

            # ---------------- input loads ----------------
            # 3 DGE issuers, each with its own descriptor ring; the 16 HW
            # DMA engines round-robin across rings (~400 GB/s aggregate).
            # Rings start ~8.6us (sync) / ~10.4us (scalar) / ~11.6us
            # (gpsimd sw-dge) after kernel start. Priority = per-ring
            # issue order, so in8 (the s1 wave) goes first on both hw
            # rings; wave-2 (xik/wlb) is interleaved in consumption order
            # behind it. gpsimd's slow ring carries only F + xts.
            in8f = in8_sb[:].rearrange("p c w -> p (c w)")
            wlbf = wlb_sb[:].rearrange("p c f -> p (c f)")
            xikf = xik8_sb[:].rearrange("p t j -> p (t j)")
            xtsf = xts_sb[:].rearrange("p c b -> p (c b)")
            SL = NCH // 12  # 6 chunks per in8 slab

            def in8_slab(si):
                cs = slice(si * SL * CW, (si + 1) * SL * CW)
                return in8f[:, cs], in8_d[:, cs]

            def xik_piece(t, q):
                js = slice(t * J + q * (J // 4),
                           t * J + (q + 1) * (J // 4))
                return xikf[:, js], xik8_d[:, js]

            def wlb_e(e):
                ws = slice(e * NCH // 8 * NO, (e + 1) * NCH // 8 * NO)
                return wlbf[:, ws], wlb_d[:, ws]

            # dma_start issue instructions are flow-controlled by ring
            # drain (~2 outstanding per ring), so a queued issue BLOCKS its
            # engine: scalar gets only the early in8 slabs (done issuing
            # before its first compute at ~20us). Wave-2 rides the sync
            # ring ALONE, strictly behind in8 in consumption order — once
            # scalar's ring drains, the lone active ring gets the full
            # ~400 GB/s, so ordering (= priority) is preserved without
            # bandwidth loss. gpsimd's slow sw-dge ring carries only F.
            # 7/5 split: sync's ring starts ~1.6us before scalar's, so it
            # carries one more slab (incl. the last-consumed one).
            for si in range(0, 10, 2):
                nc.sync.dma_start(*in8_slab(si))
                nc.scalar.dma_start(*in8_slab(si + 1))
            nc.sync.dma_start(*in8_slab(10))
            nc.sync.dma_start(*in8_slab(11))
            nc.gpsimd.dma_start(f_sb[:], f_d[:])
            for q in range(4):
                nc.sync.dma_start(*xik_piece(0, q))
                nc.sync.dma_start(*xik_piece(1, q))
                nc.sync.dma_start(*wlb_e(2 * q))
                nc.sync.dma_start(*wlb_e(2 * q + 1))
            nc.sync.dma_start(xtsf[:], xts_d[:])

            # prewarm the Exp ACT table during the DMA wait
            warm = wp.tile([128, 1], f32, name="warm", tag="warm")
            nc.vector.memset(warm[:], 0.0)
            nc.scalar.activation(warm[:], warm[:], AF.Exp)

            # fp8 copy of F (entries 0 or 2^-12: exact in e4m3) with a
            # second contraction half pointing at F again; the matching rhs
            # half is zeroed once, so the DoubleRow uv matmul computes
            # F.T @ pr with no PE perf-mode switch inside the updates.
            f8_sb = pp.tile([128, 2, 128], f8, name="f8_sb", tag="f8_sb")
            nc.vector.tensor_copy(f8_sb[:, 0, :], f_sb[:])
            nc.vector.tensor_copy(f8_sb[:, 1, :], f_sb[:])
            prb8 = pp.tile([128, 2, NCH * N_NODE], f8, name="prb8",
                           tag="prb8")
            nc.vector.memset(prb8[:, 1, :], 0.0)

            wl84 = wl8_sb.rearrange("p c (o n) -> p c o n", o=O_SZ)
            wlb4 = wlb_sb[:].rearrange("p c (o n) -> p c o n", o=O_SZ)

            # ---------------- helpers ----------------

            def dump(name, src_ap, n_cols, pdim=128):
                """Debug: convert+copy src to DRAM dump tensor."""
                if not _DEBUG or name not in dbg:
                    return
                scr = wp.tile([pdim, n_cols], f32, name="scr" + name,
                              tag="scr" + name)
                nc.vector.tensor_copy(scr[:], src_ap)
                nc.sync.dma_start(dbg[name][:], scr[:])

            def squash_half(s_src, v_out, P, nch, tag, fac_scale,
                            newton_iters=1):
                """v_out = fac_scale * squash(s_src) over o ((o,n) cols).
                rsqrt via exponent bit-trick + Newton (no ACT tables);
                pow2 fac_scale folds into the seed/last Newton constants."""
                s4 = s_src.rearrange("p c (o n) -> p c o n", o=O_SZ)
                sq = wp.tile([P, nch, NO], f32, name="sq" + tag,
                             tag="sq" + tag)
                # Square needs no ACT table load (unlike Sqrt) -> safe+free
                nc.scalar.square(sq[:], s_src)
                msq = wp.tile([P, nch, N_NODE], f32, name="msq" + tag,
                              tag="msq" + tag)
                nc.vector.reduce_sum(
                    msq[:], sq[:].rearrange("p c (o n) -> p c n o",
                                            o=O_SZ),
                    axis=AX.X)
                zi = wp.tile([P, nch, N_NODE], i32, name="zi" + tag,
                             tag="zi" + tag)
                nc.vector.tensor_scalar(
                    out=zi[:], in0=msq[:].bitcast(i32), scalar1=1,
                    scalar2=-1, op0=ALU.arith_shift_right,
                    op1=ALU.bitwise_xor)
                magic = RSQRT_MAGIC + 1
                if newton_iters == 0:
                    # fold the pow2 fac_scale into the rsqrt seed exponent
                    magic += int(np.log2(fac_scale)) << 23
                nc.vector.tensor_scalar_add(zi[:], zi[:], magic)
                z = zi[:].bitcast(f32)
                t = wp.tile([P, nch, N_NODE], f32, name="nt" + tag,
                            tag="nt" + tag)
                w = wp.tile([P, nch, N_NODE], f32, name="nw" + tag,
                            tag="nw" + tag)
                for it in range(newton_iters):
                    last = it == newton_iters - 1
                    fs = fac_scale if last else 1.0
                    nc.vector.tensor_mul(t[:], z, z)
                    nc.vector.tensor_mul(t[:], t[:], msq[:])
                    nc.vector.tensor_scalar(
                        out=w[:], in0=t[:], scalar1=-0.5 * fs,
                        scalar2=1.5 * fs, op0=ALU.mult, op1=ALU.add)
                    nc.vector.tensor_mul(z, z, w[:])
                mag = wp.tile([P, nch, N_NODE], f32, name="mag" + tag,
                              tag="mag" + tag)
                nc.vector.tensor_mul(mag[:], msq[:], z)  # fs*sqrt(msq)
                den = wp.tile([P, nch, N_NODE], f32, name="den" + tag,
                              tag="den" + tag)
                nc.vector.tensor_scalar_add(den[:], msq[:], 1.0)
                rden = wp.tile([P, nch, N_NODE], f32, name="rden" + tag,
                               tag="rden" + tag)
                nc.vector.reciprocal_approx_fast(rden[:], den[:])
                fac = wp.tile([P, nch, N_NODE], f32, name="fac" + tag,
                              tag="fac" + tag)
                nc.vector.tensor_mul(fac[:], mag[:], rden[:])
                fb = fac[:].unsqueeze(2).broadcast_to(
                    (P, nch, O_SZ, N_NODE))
                nc.vector.tensor_mul(
                    v_out.rearrange("p c (o n) -> p c o n", o=O_SZ),
                    s4, fb)

            def s_banks():
                return [ps_s.tile([128, NO], f32, name=f"s_ps{bc}",
                                  tag=f"s_ps{bc}") for bc in range(2)]

            def s_groups(bank, rhs_sb, glo, ghi):
                """s-matmul groups [glo, ghi) accumulating into bank;
                emitted in pieces so the PE streams behind the mc build."""
                for g in range(glo, ghi):
                    for bc in range(2):
                        nc.tensor.matmul(
                            bank[bc][:],
                            xt8_sb[:, 2 * g:2 * g + 2,
                                   bc * 128:(bc + 1) * 128],
                            rhs_sb[:, 2 * g:2 * g + 2, :],
                            start=(g == 0), stop=(g == NG - 1),
                            perf_mode=PM)

            def s_finish(bank, scale, v8_sb):
                s_sb = wp.tile([128, 2, NO], f32, name="s_sb", tag="s_sb")
                for bc in range(2):
                    nc.scalar.mul(s_sb[:, bc, :], bank[bc][:], scale)
                squash_half(s_sb[:], v8_sb[:], 128, 2, "m", SV,
                            newton_iters=0)
                return s_sb

            def b_update(v8_sb, first, mc_half=None, emit_q=None):
                prb = prb8[:, 0, :].rearrange("p (c n) -> p c n", n=N_NODE)
                ph = [hp.tile([128, NH, NO], bf16, name="ph",
                              tag="ph" + str(h)) for h in range(2)]
                t8s = [hp.tile([128, NH, 80], bf16, name="t8",
                               tag="t8" + str(h)) for h in range(2)]

                def q_run(h, r):
                    qrun = hp.tile([128, 9, NO], bf16, name="qrun",
                                   tag="qr" + str((h * 3 + r) % 3))
                    for gi in range(4):
                        gq = h * 12 + r * 4 + gi
                        q_ps = ps_q.tile([128, 3 * NO], f32,
                                         name="q_ps", tag="q_ps")
                        for s_i in range(3):
                            mch = gq * 3 + s_i
                            nc.tensor.matmul(
                                q_ps[:, s_i * NO:(s_i + 1) * NO],
                                xik8_sb[:, :,
                                        mch * 128:(mch + 1) * 128],
                                v8_sb[:],
                                start=True, stop=True, perf_mode=PM)
                        q3 = q_ps[:].rearrange("p (c f) -> p c f", c=3)
                        if first and h == 0 and r == 0 and gi == 0:
                            dump("d_q1", q3[:, 0, :], NO)
                        lo = (r * 4 + gi) * 3
                        if gi == 3:
                            nc.vector.tensor_mul(
                                ph[h][:, lo:lo + 3, :],
                                wlb_sb[:,
                                       h * NH + lo:h * NH + lo + 3, :],
                                q3)
                        else:
                            nc.scalar.copy(
                                qrun[:, gi * 3:gi * 3 + 3, :], q3)
                    lo = r * 12
                    nc.vector.tensor_mul(
                        ph[h][:, lo:lo + 9, :],
                        wlb_sb[:, h * NH + lo:h * NH + lo + 9, :],
                        qrun[:])
                    if first and h == 0 and r == 0:
                        dump("d_p1", ph[0][:, 0, :], NO)
                    # tree level 1 for this run's 12 chunks, pipelined so
                    # only levels 2-4 remain after the half's last multiply
                    # (GpSimd is too slow here AND its SBUF traffic slows
                    # the DVE ~1.7x — measured; keep the DVE)
                    vh = ph[h][:, lo:lo + 12, :]
                    nc.vector.tensor_add(
                        t8s[h][:, lo:lo + 12, :],
                        vh[:, :, 0:80], vh[:, :, 80:160])

                QC = NH // 2  # 18-chunk quarter

                def finish_quarter(h, q):
                    """tree l2-4 + uv + b + softmax + mc for one 18-chunk
                    quarter — fine-grained so the next iteration's
                    s-matmul starts ~4us earlier per half."""
                    qlo = h * NH + q * QC
                    qs = slice(qlo, qlo + QC)
                    t8q = t8s[h][:, q * QC:(q + 1) * QC, :]
                    t4 = hp.tile([128, QC, 40], bf16, name="t4",
                                 tag="t4" + str(h) + str(q))
                    nc.vector.tensor_add(t4[:], t8q[:, :, 0:40],
                                         t8q[:, :, 40:80])
                    t2 = hp.tile([128, QC, 20], bf16, name="t2",
                                 tag="t2" + str(h) + str(q))
                    nc.vector.tensor_add(t2[:], t4[:, :, 0:20],
                                         t4[:, :, 20:40])
                    nc.vector.tensor_add(prb[:, qs, :],
                                         t2[:, :, 0:10], t2[:, :, 10:20])
                    uv_ps = ps_f.tile([128, NH * N_NODE], f32,
                                      name=f"uv_ps{h}", tag=f"uv_ps{h}")
                    qn = QC * N_NODE
                    nc.tensor.matmul(
                        uv_ps[:, q * qn:(q + 1) * qn], f8_sb[:],
                        prb8[:, :, qlo * N_NODE:(qlo + QC) * N_NODE],
                        start=True, stop=True, perf_mode=PM)
                    uv3 = uv_ps[:, q * qn:(q + 1) * qn].rearrange(
                        "p (c n) -> p c n", n=N_NODE)
                    if first:
                        nc.scalar.copy(b_sb[:, qs, :], uv3)
                        b_src = uv3
                    else:
                        nc.vector.tensor_add(b_sb[:, qs, :],
                                             b_sb[:, qs, :], uv3)
                        b_src = b_sb[:, qs, :]
                    if first and h == 0 and q == 0:
                        dump("d_pr1", prb[:, 0, :], N_NODE)
                    softmax_part(qs, b_src)
                    if mc_half is not None:
                        mc_half(h, q)

                # Emit h1's first runs BEFORE h0's tree/uv/softmax block so
                # the in-order PE queue keeps flowing while the DVE tree
                # completes; the next iteration's s-matmul quarters stream
                # on the PE behind each finished mc quarter.
                for r in range(3):
                    q_run(0, r)
                q_run(1, 0)
                q_run(1, 1)
                finish_quarter(0, 0)
                q_run(1, 2)
                finish_quarter(0, 1)
                if emit_q is not None:
                    emit_q(0)
                    emit_q(1)
                finish_quarter(1, 0)
                if emit_q is not None:
                    emit_q(2)
                finish_quarter(1, 1)
                if emit_q is not None:
                    emit_q(3)
                return None

            e_sb = pp.tile([128, NCH, N_NODE], bf16, name="e_sb",
                           tag="e_sb")
            se = pp.tile([128, NCH], f32, name="se", tag="se")
            rse = pp.tile([128, NCH], f32, name="rse", tag="rse")
            rse_x = pp.tile([128, NCH, N_NODE], bf16, name="rse_x",
                            tag="rse_x")
            c_sb = pp.tile([128, NCH, N_NODE], bf16, name="c_sb",
                           tag="c_sb")

            def softmax_part(hs, b_src):
                ncs = hs.stop - hs.start
                nc.scalar.activation(e_sb[:, hs, :], b_src, AF.Exp)
                nc.vector.reduce_sum(se[:, hs], e_sb[:, hs, :], axis=AX.X)
                nc.vector.reciprocal_approx_fast(rse[:, hs], se[:, hs])
                # expand 1/sum to bf16 on the scalar engine so the c
                # multiply packs (2x) without extra DVE work
                nc.scalar.copy(
                    rse_x[:, hs, :],
                    rse[:, hs].unsqueeze(2).broadcast_to(
                        (128, ncs, N_NODE)))
                nc.vector.tensor_mul(c_sb[:, hs, :], e_sb[:, hs, :],
                                     rse_x[:, hs, :])

            def mc_half_fn(mc, wl4_src):
                mc4 = mc[:].rearrange("p c (o n) -> p c o n", o=O_SZ)
                cb = c_sb[:].unsqueeze(2).broadcast_to(
                    (128, NCH, O_SZ, N_NODE))

                def go(h, q):
                    # quarter (0,0) feeds the next s-matmul first -> fast
                    # DVE path; (0,1) goes to GpSimd (slow but free; its
                    # consumer is ~3us behind) as two 9-chunk ops; h1
                    # quarters stay on the DVE behind the half-1 chain.
                    qlo = (2 * h + q) * (NH // 2)
                    if (h, q) == (0, 1):
                        for half in range(2):
                            cs = slice(qlo + half * 9,
                                       qlo + (half + 1) * 9)
                            nc.gpsimd.tensor_mul(mc4[:, cs],
                                                 wl4_src[:, cs],
                                                 cb[:, cs])
                    else:
                        cs = slice(qlo, qlo + NH // 2)
                        nc.vector.tensor_mul(mc4[:, cs], wl4_src[:, cs],
                                             cb[:, cs])
                return go

            # ---------------- iteration 1 (c uniform = 0.1) ----------------
            v8 = wp.tile([128, 2, NO], f8, name="v8", tag="v8")
            bank1 = s_banks()
            s_groups(bank1, wl8_sb, 0, NG)
            s1_sb = s_finish(bank1, 0.1 / SW, v8)
            if _DEBUG:
                dump("d_s1", s1_sb[:].rearrange("p a b -> p (a b)"),
                     2 * NO)
                dump("d_v1", v8[:].rearrange("p a b -> p (a b)"), 2 * NO)
            mc8 = wp.tile([128, NCH, NO], f8, name="mc8", tag="mc8")
            bank2 = s_banks()
            b_update(v8, first=True, mc_half=mc_half_fn(mc8, wl84),
                     emit_q=lambda qi: s_groups(
                         bank2, mc8[:], qi * NG // 4, (qi + 1) * NG // 4))
            if _DEBUG:
                dump("d_b1", b_sb[:].rearrange("p a b -> p (a b)"),
                     NCH * N_NODE)
                dump("d_c1", c_sb[:].rearrange("p a b -> p (a b)"),
                     NCH * N_NODE)

            # ---------------- iteration 2 ----------------
            v8 = wp.tile([128, 2, NO], f8, name="v8b", tag="v8")
            s2_sb = s_finish(bank2, 1.0 / SW, v8)
            if _DEBUG:
                dump("d_s2", s2_sb[:].rearrange("p a b -> p (a b)"),
                     2 * NO)
            mc3 = wp.tile([128, NCH, NO], bf16, name="mc3", tag="mc3")
            s3_ps = ps_s.tile([B_SH, NO], f32, name="s3_ps",
                               tag="s_ps0")

            def s3_groups(clo, chi):
                for c in range(clo, chi):
                    nc.tensor.matmul(s3_ps[:], xts_sb[:, c, :],
                                     mc3[:, c, :],
                                     start=(c == 0), stop=(c == NCH - 1))

            b_update(v8, first=False, mc_half=mc_half_fn(mc3, wlb4),
                     emit_q=lambda qi: s3_groups(
                         qi * NCH // 4, (qi + 1) * NCH // 4))
            if _DEBUG:
                dump("d_b2", b_sb[:].rearrange("p a b -> p (a b)"),
                     NCH * N_NODE)

            # ---------------- iteration 3: bf16, own batch shard ----------
            ssh = wp.tile([B_SH, 1, NO], f32, name="ssh", tag="ssh")
            nc.scalar.copy(ssh[:, 0, :], s3_ps[:])
            if _DEBUG:
                dump("d_s3", ssh[:, 0, :], NO, pdim=B_SH)
            ysh = wp.tile([B_SH, 1, NO], f32, name="ysh", tag="ysh")
            squash_half(ssh[:], ysh[:], B_SH, 1, "s", 1.0, newton_iters=1)
            nc.scalar.dma_start(y_d[0:16, :], ysh[0:16, 0, :])
            nc.sync.dma_start(y_d[16:32, :], ysh[16:32, 0, :])

    nc.compile()
    return nc


def _pack_pm(arr2d, cols):
    """[J, cols] row-major -> [128, NCH*cols] partition-major contiguous."""
    return np.ascontiguousarray(
        arr2d.reshape(NCH, 128, cols).transpose(1, 0, 2).reshape(
            128, NCH * cols))


def _host_prep(x, W):
    """Per-core input dicts; only xts (the 32-col batch shard of x, bf16)
    differs between cores."""
    import ml_dtypes

    bf = ml_dtypes.bfloat16
    f8 = ml_dtypes.float8_e4m3
    x = np.ascontiguousarray(x, dtype=np.float32)
    W = np.ascontiguousarray(W, dtype=np.float32)
    xt = np.ascontiguousarray(x.transpose(2, 1, 0)).reshape(J, B)
    xik = np.ascontiguousarray(x.transpose(0, 2, 1)).reshape(B, J)
    # wl columns in (o, n) order: n innermost
    wl = np.ascontiguousarray(
        (np.float32(0.03) * W[0]).transpose(0, 3, 2, 1)).reshape(J, NO)
    in8 = np.concatenate(
        [(wl * np.float32(SW)).astype(f8), xt.astype(f8)], axis=1)
    in8 = _pack_pm(in8, CW)
    xik8 = np.ascontiguousarray(
        xik.astype(f8).reshape(2, 128, J).transpose(1, 0, 2).reshape(
            128, 2 * J))
    wlb = _pack_pm(wl.astype(bf), NO)
    xtb = xt.astype(bf)
    # F entries 1/(B*SV) = 2^-12: exact in bf16.
    F = (np.kron(np.eye(16, dtype=np.float32),
                 np.ones((8, 8), dtype=np.float32))
         / np.float32(B * SV)).astype(bf)
    base = {"in8": in8, "xik8": xik8, "wlb": wlb, "fmat": F}
    in_maps = []
    for c in range(N_CORES):
        m = dict(base)
        m["xts"] = _pack_pm(np.ascontiguousarray(
            xtb[:, c * B_SH:(c + 1) * B_SH]), B_SH)
        in_maps.append(m)
    return in_maps


def _run(in_maps, trace=False, all_cores=False):
    from concourse.bass_utils import run_bass_kernel_spmd

    if "nc" not in _CACHE:
        _CACHE["nc"] = _build_program()
    nc = _CACHE["nc"]
    kwargs = {}
    if all_cores:
        kwargs["trace_cores"] = list(range(N_CORES))
    res = run_bass_kernel_spmd(nc, in_maps, core_ids=list(range(N_CORES)),
                               trace=trace, **kwargs)
    return res


def kernel(x: np.ndarray, W: np.ndarray) -> np.ndarray:
    in_maps = _host_prep(x, W)
    res = _run(in_maps)
    # y columns are (o, n): reshape and swap back to (n, o)
    v = np.concatenate([res.results[c]["y"] for c in range(N_CORES)], axis=0)
    v = v.reshape(B, O_SZ, N_NODE).transpose(0, 2, 1)
    return np.ascontiguousarray(v).reshape(
        B, N_NODE, O_SZ, 1).astype(np.float32)


# revision 48
# speedup vs baseline: 1.0021x; 1.0021x over previous
"""CapsuleLayer (dynamic routing, 3 iterations) on 8 Trainium2 NeuronCores.

Zero-collective design. A collective-based kernel spends ~60us waiting for
the ncfw/TOPSP firmware to boot plus ~10us per collective; with ~25us of
real math that dominates. This kernel eliminates every collective:

  - The routing statistics (b_ij += mean over 256 batch samples of u_hat.v)
    tolerate large per-element noise (it averages out ~16x in the batch
    mean), so each core REPLICATES the full-batch routing (iterations 1-2)
    in fp8 instead of sharding it (measured ~3e-3 final rel err vs the 2e-2
    gate, identical to bf16 routing).
  - Iteration 3 (output-determining) runs in bf16 with each core producing
    only its 32-row batch shard of v_3; the host concatenates. No
    ReduceScatter, no AllGather, no warm-up, no ncfw boot.

Layout: rows j = (i,k), 9216 rows = 72 chunks of 128. All DRAM inputs are
host-packed partition-major so every DMA is contiguous. The (n,o) capsule
columns are stored (o,n) — n innermost — so every broadcast-by-n multiply
(mc = c*wl, v = fac*s) has a step-1 innermost AP on both sources and hits
the DVE 2x_1P packed mode; the o-reduction tree becomes contiguous
block-halving adds.

DMA: only sync/scalar/gpsimd can issue (hwdge + sw-dge). Issue cost is
~620ns per dma_start regardless of size, so wl8+xt8 are host-packed into
ONE tensor (in8, per-chunk [wl8|xt8]) -> 8 slab issues spread over the 3
queues, chunk-ordered so the s1 matmul streams behind the DMA wave.

Engine facts this schedule is built on (measured on this hardware):
fp8 DoubleRow matmul ~135ns pitch (256-deep, 160 free); DVE 0.54 ns/col
only when every operand is 2-byte, packed, SBUF (broadcast-over-innermost
or PSUM reads drop it to 1.07-1.37; a concurrently streaming PE degrades
DVE SBUF access up to ~2.5x); GpSimd 1.95-2.1; Scalar copy 1.2-1.4.
  s/Q matmuls   fp8 DoubleRow; rhs (wl/mc) in [p,(o,n),c] layout.
  mc = c o wl   DVE packed multiplies (c broadcast over o sits OUTSIDE the
                innermost n dim); iter-1 is fp8 (1x regardless).
  Q egress      Scalar copies 3 of 4 PSUM banks to bf16 (DVE direct-
                multiplies the 4th); DVE packed-multiplies p = wlb o Q.
  pr = sum_o p  contiguous halving adds 160->80->40->20->10, level 1
                per-run on DVE (pipelined under the next Q groups).
  uv = F.T @ pr PE; F = kron(I16, ones8x8)/(B*SV) = 2^-12 exact in bf16
                (sums k inside i-groups, replicates back, folds all scales)
  squash        scalar ACT Square + Sqrt(scale=fs^2) + DVE reduce/recip;
                fs=SV folded exactly (pow2) into the Sqrt scale.
Scales: wl8 = 16*0.03*W keeps fp8 normals; v8 = 16*v; x unscaled.
"""
import sys

if "/opt/trn_rl_repo" not in sys.path:
    sys.path.insert(0, "/opt/trn_rl_repo")

import numpy as np

N_CORES = 8
B, IN_SIZE, I_TOT = 256, 8, 1152
N_NODE, O_SZ = 10, 16
NO = N_NODE * O_SZ          # 160
J = I_TOT * IN_SIZE         # 9216 rows (i,k)
NCH = J // 128              # 72 chunks
NG = NCH // 2               # 36 DoubleRow chunk-pairs
NH = NCH // 2               # 36 chunks per b_update half
B_SH = B // N_CORES         # 32 batch rows per core
SW = 16.0                   # wl fp8 scale
SV = 16.0                   # v fp8 scale
CW = NO + B                 # 416 packed cols per chunk in in8
RSQRT_MAGIC = 0x5F3759DF

_CACHE = {}
_DEBUG = False              # adds stage-dump outputs (debugging only)


def _build_program():
    import concourse.bacc as bacc
    import concourse.tile as tile
    import concourse.mybir as mybir

    f32 = mybir.dt.float32
    bf16 = mybir.dt.bfloat16
    f8 = mybir.dt.float8e4
    i32 = mybir.dt.int32
    AF = mybir.ActivationFunctionType
    ALU = mybir.AluOpType
    AX = mybir.AxisListType
    PM = mybir.MatmulPerfMode.DoubleRow

    nc = bacc.Bacc("TRN2", target_bir_lowering=False, debug=False,
                   enable_asserts=True, num_devices=N_CORES)

    in8_d = nc.dram_tensor("in8", [128, NCH * CW], f8,
                           kind="ExternalInput").ap()
    xik8_d = nc.dram_tensor("xik8", [128, 2 * J], f8,
                            kind="ExternalInput").ap()
    wlb_d = nc.dram_tensor("wlb", [128, NCH * NO], bf16,
                           kind="ExternalInput").ap()
    xts_d = nc.dram_tensor("xts", [128, NCH * B_SH], bf16,
                           kind="ExternalInput").ap()
    f_d = nc.dram_tensor("fmat", [128, 128], bf16, kind="ExternalInput").ap()
    y_d = nc.dram_tensor("y", [B_SH, NO], f32, kind="ExternalOutput").ap()
    dbg = {}
    if _DEBUG:
        for nm, sh, dt in [("d_s1", [128, 2 * NO], f32),
                           ("d_v1", [128, 2 * NO], f32),
                           ("d_b1", [128, NCH * N_NODE], f32),
                           ("d_c1", [128, NCH * N_NODE], f32),
                           ("d_q1", [128, NO], f32),
                           ("d_p1", [128, NO], f32),
                           ("d_pr1", [128, N_NODE], f32),
                           ("d_s2", [128, 2 * NO], f32),
                           ("d_b2", [128, NCH * N_NODE], f32),
                           ("d_s3", [B_SH, NO], f32)]:
            dbg[nm] = nc.dram_tensor(nm, sh, dt,
                                     kind="ExternalOutput").ap()

    with tile.TileContext(nc) as tc:
        with tc.tile_pool(name="persist", bufs=1) as pp, \
             tc.tile_pool(name="work", bufs=1) as wp, \
             tc.tile_pool(name="half", bufs=1) as hp, \
             tc.tile_pool(name="ps_s", bufs=1, space="PSUM") as ps_s, \
             tc.tile_pool(name="ps_q", bufs=4, space="PSUM") as ps_q, \
             tc.tile_pool(name="ps_f", bufs=1, space="PSUM") as ps_f:

            in8_sb = pp.tile([128, NCH, CW], f8, name="in8_sb",
                             tag="in8_sb")
            xik8_sb = pp.tile([128, 2, J], f8, name="xik8_sb", tag="xik8_sb")
            wlb_sb = pp.tile([128, NCH, NO], bf16, name="wlb_sb",
                             tag="wlb_sb")
            xts_sb = pp.tile([128, NCH, B_SH], bf16, name="xts_sb",
                             tag="xts_sb")
            f_sb = pp.tile([128, 128], bf16, name="f_sb", tag="f_sb")
            b_sb = pp.tile([128, NCH, N_NODE], f32, name="b_sb", tag="b_sb")

            wl8_sb = in8_sb[:, :, 0:NO]
            xt8_sb = in8_sb[:, :, NO:CW]

            # ---------------- input loads ----------------
            # 3 DGE issuers, each with its own descriptor ring; the 16 HW
            # DMA engines round-robin across rings (~400 GB/s aggregate).
            # Rings start ~8.6us (sync) / ~10.4us (scalar) / ~11.6us
            # (gpsimd sw-dge) after kernel start. Priority = per-ring
            # issue order, so in8 (the s1 wave) goes first on both hw
            # rings; wave-2 (xik/wlb) is interleaved in consumption order
            # behind it. gpsimd's slow ring carries only F + xts.
            in8f = in8_sb[:].rearrange("p c w -> p (c w)")
            wlbf = wlb_sb[:].rearrange("p c f -> p (c f)")
            xikf = xik8_sb[:].rearrange("p t j -> p (t j)")
            xtsf = xts_sb[:].rearrange("p c b -> p (c b)")
            SL = NCH // 12  # 6 chunks per in8 slab

            def in8_slab(si):
                cs = slice(si * SL * CW, (si + 1) * SL * CW)
                return in8f[:, cs], in8_d[:, cs]

            def xik_piece(t, q):
                js = slice(t * J + q * (J // 4),
                           t * J + (q + 1) * (J // 4))
                return xikf[:, js], xik8_d[:, js]

            def wlb_e(e):
                ws = slice(e * NCH // 8 * NO, (e + 1) * NCH // 8 * NO)
                return wlbf[:, ws], wlb_d[:, ws]

            # dma_start issue instructions are flow-controlled by ring
            # drain (~2 outstanding per ring), so a queued issue BLOCKS its
            # engine: scalar gets only the early in8 slabs (done issuing
            # before its first compute at ~20us). Wave-2 rides the sync
            # ring ALONE, strictly behind in8 in consumption order — once
            # scalar's ring drains, the lone active ring gets the full
            # ~400 GB/s, so ordering (= priority) is preserved without
            # bandwidth loss. gpsimd's slow sw-dge ring carries only F.
            # 7/5 split: sync's ring starts ~1.6us before scalar's, so it
            # carries one more slab (incl. the last-consumed one).
            for si in range(0, 10, 2):
                nc.sync.dma_start(*in8_slab(si))
                nc.scalar.dma_start(*in8_slab(si + 1))
            nc.sync.dma_start(*in8_slab(10))
            nc.sync.dma_start(*in8_slab(11))
            nc.gpsimd.dma_start(f_sb[:], f_d[:])
            for q in range(4):
                nc.sync.dma_start(*xik_piece(0, q))
                nc.sync.dma_start(*xik_piece(1, q))
                nc.sync.dma_start(*wlb_e(2 * q))
                nc.sync.dma_start(*wlb_e(2 * q + 1))
            nc.sync.dma_start(xtsf[:], xts_d[:])

            # prewarm the Exp ACT table during the DMA wait
            warm = wp.tile([128, 1], f32, name="warm", tag="warm")
            nc.vector.memset(warm[:], 0.0)
            nc.scalar.activation(warm[:], warm[:], AF.Exp)

            # fp8 copy of F (entries 0 or 2^-12: exact in e4m3) with a
            # second contraction half pointing at F again; the matching rhs
            # half is zeroed once, so the DoubleRow uv matmul computes
            # F.T @ pr with no PE perf-mode switch inside the updates.
            f8_sb = pp.tile([128, 2, 128], f8, name="f8_sb", tag="f8_sb")
            nc.vector.tensor_copy(f8_sb[:, 0, :], f_sb[:])
            nc.vector.tensor_copy(f8_sb[:, 1, :], f_sb[:])
            prb8 = pp.tile([128, 2, NCH * N_NODE], f8, name="prb8",
                           tag="prb8")
            nc.vector.memset(prb8[:, 1, :], 0.0)

            wl84 = wl8_sb.rearrange("p c (o n) -> p c o n", o=O_SZ)
            wlb4 = wlb_sb[:].rearrange("p c (o n) -> p c o n", o=O_SZ)

            # ---------------- helpers ----------------

            def dump(name, src_ap, n_cols, pdim=128):
                """Debug: convert+copy src to DRAM dump tensor."""
                if not _DEBUG or name not in dbg:
                    return
                scr = wp.tile([pdim, n_cols], f32, name="scr" + name,
                              tag="scr" + name)
                nc.vector.tensor_copy(scr[:], src_ap)
                nc.sync.dma_start(dbg[name][:], scr[:])

            def squash_half(s_src, v_out, P, nch, tag, fac_scale,
                            newton_iters=1):
                """v_out = fac_scale * squash(s_src) over o ((o,n) cols).
                rsqrt via exponent bit-trick + Newton (no ACT tables);
                pow2 fac_scale folds into the seed/last Newton constants."""
                s4 = s_src.rearrange("p c (o n) -> p c o n", o=O_SZ)
                sq = wp.tile([P, nch, NO], f32, name="sq" + tag,
                             tag="sq" + tag)
                # Square needs no ACT table load (unlike Sqrt) -> safe+free
                nc.scalar.square(sq[:], s_src)
                msq = wp.tile([P, nch, N_NODE], f32, name="msq" + tag,
                              tag="msq" + tag)
                nc.vector.reduce_sum(
                    msq[:], sq[:].rearrange("p c (o n) -> p c n o",
                                            o=O_SZ),
                    axis=AX.X)
                zi = wp.tile([P, nch, N_NODE], i32, name="zi" + tag,
                             tag="zi" + tag)
                nc.vector.tensor_scalar(
                    out=zi[:], in0=msq[:].bitcast(i32), scalar1=1,
                    scalar2=-1, op0=ALU.arith_shift_right,
                    op1=ALU.bitwise_xor)
                magic = RSQRT_MAGIC + 1
                if newton_iters == 0:
                    # fold the pow2 fac_scale into the rsqrt seed exponent
                    magic += int(np.log2(fac_scale)) << 23
                nc.vector.tensor_scalar_add(zi[:], zi[:], magic)
                z = zi[:].bitcast(f32)
                t = wp.tile([P, nch, N_NODE], f32, name="nt" + tag,
                            tag="nt" + tag)
                w = wp.tile([P, nch, N_NODE], f32, name="nw" + tag,
                            tag="nw" + tag)
                for it in range(newton_iters):
                    last = it == newton_iters - 1
                    fs = fac_scale if last else 1.0
                    nc.vector.tensor_mul(t[:], z, z)
                    nc.vector.tensor_mul(t[:], t[:], msq[:])
                    nc.vector.tensor_scalar(
                        out=w[:], in0=t[:], scalar1=-0.5 * fs,
                        scalar2=1.5 * fs, op0=ALU.mult, op1=ALU.add)
                    nc.vector.tensor_mul(z, z, w[:])
                mag = wp.tile([P, nch, N_NODE], f32, name="mag" + tag,
                              tag="mag" + tag)
                nc.vector.tensor_mul(mag[:], msq[:], z)  # fs*sqrt(msq)
                den = wp.tile([P, nch, N_NODE], f32, name="den" + tag,
                              tag="den" + tag)
                nc.vector.tensor_scalar_add(den[:], msq[:], 1.0)
                rden = wp.tile([P, nch, N_NODE], f32, name="rden" + tag,
                               tag="rden" + tag)
                nc.vector.reciprocal_approx_fast(rden[:], den[:])
                fac = wp.tile([P, nch, N_NODE], f32, name="fac" + tag,
                              tag="fac" + tag)
                nc.vector.tensor_mul(fac[:], mag[:], rden[:])
                fb = fac[:].unsqueeze(2).broadcast_to(
                    (P, nch, O_SZ, N_NODE))
                nc.vector.tensor_mul(
                    v_out.rearrange("p c (o n) -> p c o n", o=O_SZ),
                    s4, fb)

            def s_banks():
                return [ps_s.tile([128, NO], f32, name=f"s_ps{bc}",
                                  tag=f"s_ps{bc}") for bc in range(2)]

            def s_groups(bank, rhs_sb, glo, ghi):
                """s-matmul groups [glo, ghi) accumulating into bank;
                emitted in pieces so the PE streams behind the mc build."""
                for g in range(glo, ghi):
                    for bc in range(2):
                        nc.tensor.matmul(
                            bank[bc][:],
                            xt8_sb[:, 2 * g:2 * g + 2,
                                   bc * 128:(bc + 1) * 128],
                            rhs_sb[:, 2 * g:2 * g + 2, :],
                            start=(g == 0), stop=(g == NG - 1),
                            perf_mode=PM)

            def s_finish(bank, scale, v8_sb):
                s_sb = wp.tile([128, 2, NO], f32, name="s_sb", tag="s_sb")
                for bc in range(2):
                    nc.scalar.mul(s_sb[:, bc, :], bank[bc][:], scale)
                squash_half(s_sb[:], v8_sb[:], 128, 2, "m", SV,
                            newton_iters=0)
                return s_sb

            def b_update(v8_sb, first, mc_half=None, emit_q=None):
                prb = prb8[:, 0, :].rearrange("p (c n) -> p c n", n=N_NODE)
                ph = [hp.tile([128, NH, NO], bf16, name="ph",
                              tag="ph" + str(h)) for h in range(2)]
                t8s = [hp.tile([128, NH, 80], bf16, name="t8",
                               tag="t8" + str(h)) for h in range(2)]

                def q_run(h, r):
                    qrun = hp.tile([128, 9, NO], bf16, name="qrun",
                                   tag="qr" + str((h * 3 + r) % 3))
                    for gi in range(4):
                        gq = h * 12 + r * 4 + gi
                        q_ps = ps_q.tile([128, 3 * NO], f32,
                                         name="q_ps", tag="q_ps")
                        for s_i in range(3):
                            mch = gq * 3 + s_i
                            nc.tensor.matmul(
                                q_ps[:, s_i * NO:(s_i + 1) * NO],
                                xik8_sb[:, :,
                                        mch * 128:(mch + 1) * 128],
                                v8_sb[:],
                                start=True, stop=True, perf_mode=PM)
                        q3 = q_ps[:].rearrange("p (c f) -> p c f", c=3)
                        if first and h == 0 and r == 0 and gi == 0:
                            dump("d_q1", q3[:, 0, :], NO)
                        lo = (r * 4 + gi) * 3
                        if gi == 3:
                            nc.vector.tensor_mul(
                                ph[h][:, lo:lo + 3, :],
                                wlb_sb[:,
                                       h * NH + lo:h * NH + lo + 3, :],
                                q3)
                        else:
                            nc.scalar.copy(
                                qrun[:, gi * 3:gi * 3 + 3, :], q3)
                    lo = r * 12
                    nc.vector.tensor_mul(
                        ph[h][:, lo:lo + 9, :],
                        wlb_sb[:, h * NH + lo:h * NH + lo + 9, :],
                        qrun[:])
                    if first and h == 0 and r == 0:
                        dump("d_p1", ph[0][:, 0, :], NO)
                    # tree level 1 for this run's 12 chunks, pipelined so
                    # only levels 2-4 remain after the half's last multiply
                    # (GpSimd is too slow here AND its SBUF traffic slows
                    # the DVE ~1.7x — measured; keep the DVE)
                    vh = ph[h][:, lo:lo + 12, :]
                    nc.vector.tensor_add(
                        t8s[h][:, lo:lo + 12, :],
                        vh[:, :, 0:80], vh[:, :, 80:160])

                def finish_half(h):
                    hs = slice(h * NH, (h + 1) * NH)
                    t8 = t8s[h]
                    t4 = hp.tile([128, NH, 40], bf16, name="t4",
                                 tag="t4" + str(h))
                    nc.vector.tensor_add(t4[:], t8[:, :, 0:40],
                                         t8[:, :, 40:80])
                    t2 = hp.tile([128, NH, 20], bf16, name="t2",
                                 tag="t2" + str(h))
                    nc.vector.tensor_add(t2[:], t4[:, :, 0:20],
                                         t4[:, :, 20:40])
                    nc.vector.tensor_add(prb[:, hs, :],
                                         t2[:, :, 0:10], t2[:, :, 10:20])
                    uv_ps = ps_f.tile([128, NH * N_NODE], f32,
                                      name=f"uv_ps{h}", tag=f"uv_ps{h}")
                    W2 = NH * N_NODE
                    nc.tensor.matmul(
                        uv_ps[:], f8_sb[:],
                        prb8[:, :, h * W2:(h + 1) * W2],
                        start=True, stop=True, perf_mode=PM)
                    uv3 = uv_ps[:].rearrange("p (c n) -> p c n", n=N_NODE)
                    if first:
                        nc.scalar.copy(b_sb[:, hs, :], uv3)
                        b_src = uv3
                    else:
                        nc.vector.tensor_add(b_sb[:, hs, :],
                                             b_sb[:, hs, :], uv3)
                        b_src = b_sb[:, hs, :]
                    if first and h == 0:
                        dump("d_pr1", prb[:, 0, :], N_NODE)
                    softmax_part(hs, b_src)
                    if mc_half is not None:
                        mc_half(h)

                # Emit h1's first runs BEFORE h0's tree/uv/softmax block so
                # the in-order PE queue keeps flowing while the DVE tree
                # completes; the next iteration's s-matmul halves stream
                # on the PE behind the finished mc halves.
                for r in range(3):
                    q_run(0, r)
                q_run(1, 0)
                q_run(1, 1)
                finish_half(0)
                q_run(1, 2)
                if emit_q is not None:
                    emit_q(0)
                    emit_q(1)
                finish_half(1)
                if emit_q is not None:
                    emit_q(2)
                    emit_q(3)
                return None

            e_sb = pp.tile([128, NCH, N_NODE], bf16, name="e_sb",
                           tag="e_sb")
            se = pp.tile([128, NCH], f32, name="se", tag="se")
            rse = pp.tile([128, NCH], f32, name="rse", tag="rse")
            rse_x = pp.tile([128, NCH, N_NODE], bf16, name="rse_x",
                            tag="rse_x")
            c_sb = pp.tile([128, NCH, N_NODE], bf16, name="c_sb",
                           tag="c_sb")

            def softmax_part(hs, b_src):
                ncs = hs.stop - hs.start
                nc.scalar.activation(e_sb[:, hs, :], b_src, AF.Exp)
                nc.vector.reduce_sum(se[:, hs], e_sb[:, hs, :], axis=AX.X)
                nc.vector.reciprocal_approx_fast(rse[:, hs], se[:, hs])
                # expand 1/sum to bf16 on the scalar engine so the c
                # multiply packs (2x) without extra DVE work
                nc.scalar.copy(
                    rse_x[:, hs, :],
                    rse[:, hs].unsqueeze(2).broadcast_to(
                        (128, ncs, N_NODE)))
                nc.vector.tensor_mul(c_sb[:, hs, :], e_sb[:, hs, :],
                                     rse_x[:, hs, :])

            def mc_half_fn(mc, wl4_src):
                mc4 = mc[:].rearrange("p c (o n) -> p c o n", o=O_SZ)
                cb = c_sb[:].unsqueeze(2).broadcast_to(
                    (128, NCH, O_SZ, N_NODE))

                def go(h):
                    # slab 0 is the first the next s-matmul consumes ->
                    # fast DVE path; GpSimd (slow but free) covers 1,2
                    # which the PE reaches only ~3.2/4.8us later. h1's
                    # slabs go on the DVE after the half-1 chain.
                    if h == 0:
                        slabs = [(0, nc.vector), (1, nc.gpsimd),
                                 (2, nc.gpsimd)]
                    else:
                        slabs = [(3, nc.vector), (4, nc.vector),
                                 (5, nc.vector)]
                    for sl, eng in slabs:
                        cs = slice(sl * 12, (sl + 1) * 12)
                        eng.tensor_mul(mc4[:, cs], wl4_src[:, cs],
                                       cb[:, cs])
                return go

            # ---------------- iteration 1 (c uniform = 0.1) ----------------
            v8 = wp.tile([128, 2, NO], f8, name="v8", tag="v8")
            bank1 = s_banks()
            s_groups(bank1, wl8_sb, 0, NG)
            s1_sb = s_finish(bank1, 0.1 / SW, v8)
            if _DEBUG:
                dump("d_s1", s1_sb[:].rearrange("p a b -> p (a b)"),
                     2 * NO)
                dump("d_v1", v8[:].rearrange("p a b -> p (a b)"), 2 * NO)
            mc8 = wp.tile([128, NCH, NO], f8, name="mc8", tag="mc8")
            bank2 = s_banks()
            b_update(v8, first=True, mc_half=mc_half_fn(mc8, wl84),
                     emit_q=lambda qi: s_groups(
                         bank2, mc8[:], qi * NG // 4, (qi + 1) * NG // 4))
            if _DEBUG:
                dump("d_b1", b_sb[:].rearrange("p a b -> p (a b)"),
                     NCH * N_NODE)
                dump("d_c1", c_sb[:].rearrange("p a b -> p (a b)"),
                     NCH * N_NODE)

            # ---------------- iteration 2 ----------------
            v8 = wp.tile([128, 2, NO], f8, name="v8b", tag="v8")
            s2_sb = s_finish(bank2, 1.0 / SW, v8)
            if _DEBUG:
                dump("d_s2", s2_sb[:].rearrange("p a b -> p (a b)"),
                     2 * NO)
            mc3 = wp.tile([128, NCH, NO], bf16, name="mc3", tag="mc3")
            s3_ps = ps_s.tile([B_SH, NO], f32, name="s3_ps",
                               tag="s_ps0")

            def s3_groups(clo, chi):
                for c in range(clo, chi):
                    nc.tensor.matmul(s3_ps[:], xts_sb[:, c, :],
                                     mc3[:, c, :],
                                     start=(c == 0), stop=(c == NCH - 1))

            b_update(v8, first=False, mc_half=mc_half_fn(mc3, wlb4),
                     emit_q=lambda qi: s3_groups(
                         qi * NCH // 4, (qi + 1) * NCH // 4))
            if _DEBUG:
                dump("d_b2", b_sb[:].rearrange("p a b -> p (a b)"),
                     NCH * N_NODE)

            # ---------------- iteration 3: bf16, own batch shard ----------
            ssh = wp.tile([B_SH, 1, NO], f32, name="ssh", tag="ssh")
            nc.scalar.copy(ssh[:, 0, :], s3_ps[:])
            if _DEBUG:
                dump("d_s3", ssh[:, 0, :], NO, pdim=B_SH)
            ysh = wp.tile([B_SH, 1, NO], f32, name="ysh", tag="ysh")
            squash_half(ssh[:], ysh[:], B_SH, 1, "s", 1.0, newton_iters=1)
            nc.scalar.dma_start(y_d[0:16, :], ysh[0:16, 0, :])
            nc.sync.dma_start(y_d[16:32, :], ysh[16:32, 0, :])

    nc.compile()
    return nc


def _pack_pm(arr2d, cols):
    """[J, cols] row-major -> [128, NCH*cols] partition-major contiguous."""
    return np.ascontiguousarray(
        arr2d.reshape(NCH, 128, cols).transpose(1, 0, 2).reshape(
            128, NCH * cols))


def _host_prep(x, W):
    """Per-core input dicts; only xts (the 32-col batch shard of x, bf16)
    differs between cores."""
    import ml_dtypes

    bf = ml_dtypes.bfloat16
    f8 = ml_dtypes.float8_e4m3
    x = np.ascontiguousarray(x, dtype=np.float32)
    W = np.ascontiguousarray(W, dtype=np.float32)
    xt = np.ascontiguousarray(x.transpose(2, 1, 0)).reshape(J, B)
    xik = np.ascontiguousarray(x.transpose(0, 2, 1)).reshape(B, J)
    # wl columns in (o, n) order: n innermost
    wl = np.ascontiguousarray(
        (np.float32(0.03) * W[0]).transpose(0, 3, 2, 1)).reshape(J, NO)
    in8 = np.concatenate(
        [(wl * np.float32(SW)).astype(f8), xt.astype(f8)], axis=1)
    in8 = _pack_pm(in8, CW)
    xik8 = np.ascontiguousarray(
        xik.astype(f8).reshape(2, 128, J).transpose(1, 0, 2).reshape(
            128, 2 * J))
    wlb = _pack_pm(wl.astype(bf), NO)
    xtb = xt.astype(bf)
    # F entries 1/(B*SV) = 2^-12: exact in bf16.
    F = (np.kron(np.eye(16, dtype=np.float32),
                 np.ones((8, 8), dtype=np.float32))
         / np.float32(B * SV)).astype(bf)
    base = {"in8": in8, "xik8": xik8, "wlb": wlb, "fmat": F}
    in_maps = []
    for c in range(N_CORES):
        m = dict(base)
        m["xts"] = _pack_pm(np.ascontiguousarray(
            xtb[:, c * B_SH:(c + 1) * B_SH]), B_SH)
        in_maps.append(m)
    return in_maps


def _run(in_maps, trace=False, all_cores=False):
    from concourse.bass_utils import run_bass_kernel_spmd

    if "nc" not in _CACHE:
        _CACHE["nc"] = _build_program()
    nc = _CACHE["nc"]
    kwargs = {}
    if all_cores:
        kwargs["trace_cores"] = list(range(N_CORES))
    res = run_bass_kernel_spmd(nc, in_maps, core_ids=list(range(N_CORES)),
                               trace=trace, **kwargs)
    return res


def kernel(x: np.ndarray, W: np.ndarray) -> np.ndarray:
    in_maps = _host_prep(x, W)
    res = _run(in_maps)
    # y columns are (o, n): reshape and swap back to (n, o)
    v = np.concatenate([res.results[c]["y"] for c in range(N_CORES)], axis=0)
    v = v.reshape(B, O_SZ, N_NODE).transpose(0, 2, 1)
    return np.ascontiguousarray(v).reshape(
        B, N_NODE, O_SZ, 1).astype(np.float32)


# revision 49
# speedup vs baseline: 1.0206x; 1.0184x over previous
"""CapsuleLayer (dynamic routing, 3 iterations) on 8 Trainium2 NeuronCores.

Zero-collective design. A collective-based kernel spends ~60us waiting for
the ncfw/TOPSP firmware to boot plus ~10us per collective; with ~25us of
real math that dominates. This kernel eliminates every collective:

  - The routing statistics (b_ij += mean over 256 batch samples of u_hat.v)
    tolerate large per-element noise (it averages out ~16x in the batch
    mean), so each core REPLICATES the full-batch routing (iterations 1-2)
    in fp8 instead of sharding it (measured ~3e-3 final rel err vs the 2e-2
    gate, identical to bf16 routing).
  - Iteration 3 (output-determining) runs in bf16 with each core producing
    only its 32-row batch shard of v_3; the host concatenates. No
    ReduceScatter, no AllGather, no warm-up, no ncfw boot.

Layout: rows j = (i,k), 9216 rows = 72 chunks of 128. All DRAM inputs are
host-packed partition-major so every DMA is contiguous. The (n,o) capsule
columns are stored (o,n) — n innermost — so every broadcast-by-n multiply
(mc = c*wl, v = fac*s) has a step-1 innermost AP on both sources and hits
the DVE 2x_1P packed mode; the o-reduction tree becomes contiguous
block-halving adds.

DMA: only sync/scalar/gpsimd can issue (hwdge + sw-dge). Issue cost is
~620ns per dma_start regardless of size, so wl8+xt8 are host-packed into
ONE tensor (in8, per-chunk [wl8|xt8]) -> 8 slab issues spread over the 3
queues, chunk-ordered so the s1 matmul streams behind the DMA wave.

Engine facts this schedule is built on (measured on this hardware):
fp8 DoubleRow matmul ~135ns pitch (256-deep, 160 free); DVE 0.54 ns/col
only when every operand is 2-byte, packed, SBUF (broadcast-over-innermost
or PSUM reads drop it to 1.07-1.37; a concurrently streaming PE degrades
DVE SBUF access up to ~2.5x); GpSimd 1.95-2.1; Scalar copy 1.2-1.4.
  s/Q matmuls   fp8 DoubleRow; rhs (wl/mc) in [p,(o,n),c] layout.
  mc = c o wl   DVE packed multiplies (c broadcast over o sits OUTSIDE the
                innermost n dim); iter-1 is fp8 (1x regardless).
  Q egress      Scalar copies 3 of 4 PSUM banks to bf16 (DVE direct-
                multiplies the 4th); DVE packed-multiplies p = wlb o Q.
  pr = sum_o p  contiguous halving adds 160->80->40->20->10, level 1
                per-run on DVE (pipelined under the next Q groups).
  uv = F.T @ pr PE; F = kron(I16, ones8x8)/(B*SV) = 2^-12 exact in bf16
                (sums k inside i-groups, replicates back, folds all scales)
  squash        scalar ACT Square + Sqrt(scale=fs^2) + DVE reduce/recip;
                fs=SV folded exactly (pow2) into the Sqrt scale.
Scales: wl8 = 16*0.03*W keeps fp8 normals; v8 = 16*v; x unscaled.
"""
import sys

if "/opt/trn_rl_repo" not in sys.path:
    sys.path.insert(0, "/opt/trn_rl_repo")

import numpy as np

N_CORES = 8
B, IN_SIZE, I_TOT = 256, 8, 1152
N_NODE, O_SZ = 10, 16
NO = N_NODE * O_SZ          # 160
J = I_TOT * IN_SIZE         # 9216 rows (i,k)
NCH = J // 128              # 72 chunks
NG = NCH // 2               # 36 DoubleRow chunk-pairs
NH = NCH // 2               # 36 chunks per b_update half
B_SH = B // N_CORES         # 32 batch rows per core
SW = 16.0                   # wl fp8 scale
SV = 16.0                   # v fp8 scale
CW = NO + B                 # 416 packed cols per chunk in in8
RSQRT_MAGIC = 0x5F3759DF

_CACHE = {}
_DEBUG = False              # adds stage-dump outputs (debugging only)


def _build_program():
    import concourse.bacc as bacc
    import concourse.tile as tile
    import concourse.mybir as mybir

    f32 = mybir.dt.float32
    bf16 = mybir.dt.bfloat16
    f8 = mybir.dt.float8e4
    i32 = mybir.dt.int32
    AF = mybir.ActivationFunctionType
    ALU = mybir.AluOpType
    AX = mybir.AxisListType
    PM = mybir.MatmulPerfMode.DoubleRow

    nc = bacc.Bacc("TRN2", target_bir_lowering=False, debug=False,
                   enable_asserts=True, num_devices=N_CORES)

    in8_d = nc.dram_tensor("in8", [128, NCH * CW], f8,
                           kind="ExternalInput").ap()
    xik8_d = nc.dram_tensor("xik8", [128, 2 * J], f8,
                            kind="ExternalInput").ap()
    wlb_d = nc.dram_tensor("wlb", [128, NCH * NO], bf16,
                           kind="ExternalInput").ap()
    xts_d = nc.dram_tensor("xts", [128, NCH * B_SH], bf16,
                           kind="ExternalInput").ap()
    f_d = nc.dram_tensor("fmat", [128, 128], bf16, kind="ExternalInput").ap()
    y_d = nc.dram_tensor("y", [B_SH, NO], f32, kind="ExternalOutput").ap()
    dbg = {}
    if _DEBUG:
        for nm, sh, dt in [("d_s1", [128, 2 * NO], f32),
                           ("d_v1", [128, 2 * NO], f32),
                           ("d_b1", [128, NCH * N_NODE], f32),
                           ("d_c1", [128, NCH * N_NODE], f32),
                           ("d_q1", [128, NO], f32),
                           ("d_p1", [128, NO], f32),
                           ("d_pr1", [128, N_NODE], f32),
                           ("d_s2", [128, 2 * NO], f32),
                           ("d_b2", [128, NCH * N_NODE], f32),
                           ("d_s3", [B_SH, NO], f32)]:
            dbg[nm] = nc.dram_tensor(nm, sh, dt,
                                     kind="ExternalOutput").ap()

    with tile.TileContext(nc) as tc:
        with tc.tile_pool(name="persist", bufs=1) as pp, \
             tc.tile_pool(name="work", bufs=1) as wp, \
             tc.tile_pool(name="half", bufs=1) as hp, \
             tc.tile_pool(name="ps_s", bufs=1, space="PSUM") as ps_s, \
             tc.tile_pool(name="ps_q", bufs=4, space="PSUM") as ps_q, \
             tc.tile_pool(name="ps_f", bufs=1, space="PSUM") as ps_f:

            in8_sb = pp.tile([128, NCH, CW], f8, name="in8_sb",
                             tag="in8_sb")
            xik8_sb = pp.tile([128, 2, J], f8, name="xik8_sb", tag="xik8_sb")
            wlb_sb = pp.tile([128, NCH, NO], bf16, name="wlb_sb",
                             tag="wlb_sb")
            xts_sb = pp.tile([128, NCH, B_SH], bf16, name="xts_sb",
                             tag="xts_sb")
            f_sb = pp.tile([128, 128], bf16, name="f_sb", tag="f_sb")
            b_sb = pp.tile([128, NCH, N_NODE], f32, name="b_sb", tag="b_sb")

            wl8_sb = in8_sb[:, :, 0:NO]
            xt8_sb = in8_sb[:, :, NO:CW]

            # ---------------- input loads ----------------
            # 3 DGE issuers, each with its own descriptor ring; the 16 HW
            # DMA engines round-robin across rings (~400 GB/s aggregate).
            # Rings start ~8.6us (sync) / ~10.4us (scalar) / ~11.6us
            # (gpsimd sw-dge) after kernel start. Priority = per-ring
            # issue order, so in8 (the s1 wave) goes first on both hw
            # rings; wave-2 (xik/wlb) is interleaved in consumption order
            # behind it. gpsimd's slow ring carries only F + xts.
            in8f = in8_sb[:].rearrange("p c w -> p (c w)")
            wlbf = wlb_sb[:].rearrange("p c f -> p (c f)")
            xikf = xik8_sb[:].rearrange("p t j -> p (t j)")
            xtsf = xts_sb[:].rearrange("p c b -> p (c b)")
            SL = NCH // 12  # 6 chunks per in8 slab

            def in8_slab(si):
                cs = slice(si * SL * CW, (si + 1) * SL * CW)
                return in8f[:, cs], in8_d[:, cs]

            def xik_piece(t, q):
                js = slice(t * J + q * (J // 4),
                           t * J + (q + 1) * (J // 4))
                return xikf[:, js], xik8_d[:, js]

            def wlb_e(e):
                ws = slice(e * NCH // 8 * NO, (e + 1) * NCH // 8 * NO)
                return wlbf[:, ws], wlb_d[:, ws]

            # dma_start issue instructions are flow-controlled by ring
            # drain (~2 outstanding per ring), so a queued issue BLOCKS its
            # engine: scalar gets only the early in8 slabs (done issuing
            # before its first compute at ~20us). Wave-2 rides the sync
            # ring ALONE, strictly behind in8 in consumption order — once
            # scalar's ring drains, the lone active ring gets the full
            # ~400 GB/s, so ordering (= priority) is preserved without
            # bandwidth loss. gpsimd's slow sw-dge ring carries only F.
            for si in range(0, 12, 2):
                nc.sync.dma_start(*in8_slab(si))
                nc.scalar.dma_start(*in8_slab(si + 1))
            nc.gpsimd.dma_start(f_sb[:], f_d[:])
            for q in range(4):
                nc.sync.dma_start(*xik_piece(0, q))
                nc.sync.dma_start(*xik_piece(1, q))
                nc.sync.dma_start(*wlb_e(2 * q))
                nc.sync.dma_start(*wlb_e(2 * q + 1))
            nc.sync.dma_start(xtsf[:], xts_d[:])

            # prewarm the Exp ACT table during the DMA wait
            warm = wp.tile([128, 1], f32, name="warm", tag="warm")
            nc.vector.memset(warm[:], 0.0)
            nc.scalar.activation(warm[:], warm[:], AF.Exp)

            # fp8 copy of F (entries 0 or 2^-12: exact in e4m3) with a
            # second contraction half pointing at F again; the matching rhs
            # half is zeroed once, so the DoubleRow uv matmul computes
            # F.T @ pr with no PE perf-mode switch inside the updates.
            f8_sb = pp.tile([128, 2, 128], f8, name="f8_sb", tag="f8_sb")
            nc.vector.tensor_copy(f8_sb[:, 0, :], f_sb[:])
            nc.vector.tensor_copy(f8_sb[:, 1, :], f_sb[:])
            prb8 = pp.tile([128, 2, NCH * N_NODE], f8, name="prb8",
                           tag="prb8")
            nc.vector.memset(prb8[:, 1, :], 0.0)

            wl84 = wl8_sb.rearrange("p c (o n) -> p c o n", o=O_SZ)
            wlb4 = wlb_sb[:].rearrange("p c (o n) -> p c o n", o=O_SZ)

            # ---------------- helpers ----------------

            def dump(name, src_ap, n_cols, pdim=128):
                """Debug: convert+copy src to DRAM dump tensor."""
                if not _DEBUG or name not in dbg:
                    return
                scr = wp.tile([pdim, n_cols], f32, name="scr" + name,
                              tag="scr" + name)
                nc.vector.tensor_copy(scr[:], src_ap)
                nc.sync.dma_start(dbg[name][:], scr[:])

            def squash_half(s_src, v_out, P, nch, tag, fac_scale,
                            newton_iters=1):
                """v_out = fac_scale * squash(s_src) over o ((o,n) cols).
                rsqrt via exponent bit-trick + Newton (no ACT tables);
                pow2 fac_scale folds into the seed/last Newton constants."""
                s4 = s_src.rearrange("p c (o n) -> p c o n", o=O_SZ)
                sq = wp.tile([P, nch, NO], f32, name="sq" + tag,
                             tag="sq" + tag)
                # Square needs no ACT table load (unlike Sqrt) -> safe+free
                nc.scalar.square(sq[:], s_src)
                msq = wp.tile([P, nch, N_NODE], f32, name="msq" + tag,
                              tag="msq" + tag)
                nc.vector.reduce_sum(
                    msq[:], sq[:].rearrange("p c (o n) -> p c n o",
                                            o=O_SZ),
                    axis=AX.X)
                zi = wp.tile([P, nch, N_NODE], i32, name="zi" + tag,
                             tag="zi" + tag)
                nc.vector.tensor_scalar(
                    out=zi[:], in0=msq[:].bitcast(i32), scalar1=1,
                    scalar2=-1, op0=ALU.arith_shift_right,
                    op1=ALU.bitwise_xor)
                magic = RSQRT_MAGIC + 1
                if newton_iters == 0:
                    # fold the pow2 fac_scale into the rsqrt seed exponent
                    magic += int(np.log2(fac_scale)) << 23
                nc.vector.tensor_scalar_add(zi[:], zi[:], magic)
                z = zi[:].bitcast(f32)
                t = wp.tile([P, nch, N_NODE], f32, name="nt" + tag,
                            tag="nt" + tag)
                w = wp.tile([P, nch, N_NODE], f32, name="nw" + tag,
                            tag="nw" + tag)
                for it in range(newton_iters):
                    last = it == newton_iters - 1
                    fs = fac_scale if last else 1.0
                    nc.vector.tensor_mul(t[:], z, z)
                    nc.vector.tensor_mul(t[:], t[:], msq[:])
                    nc.vector.tensor_scalar(
                        out=w[:], in0=t[:], scalar1=-0.5 * fs,
                        scalar2=1.5 * fs, op0=ALU.mult, op1=ALU.add)
                    nc.vector.tensor_mul(z, z, w[:])
                mag = wp.tile([P, nch, N_NODE], f32, name="mag" + tag,
                              tag="mag" + tag)
                nc.vector.tensor_mul(mag[:], msq[:], z)  # fs*sqrt(msq)
                den = wp.tile([P, nch, N_NODE], f32, name="den" + tag,
                              tag="den" + tag)
                nc.vector.tensor_scalar_add(den[:], msq[:], 1.0)
                rden = wp.tile([P, nch, N_NODE], f32, name="rden" + tag,
                               tag="rden" + tag)
                nc.vector.reciprocal_approx_fast(rden[:], den[:])
                fac = wp.tile([P, nch, N_NODE], f32, name="fac" + tag,
                              tag="fac" + tag)
                nc.vector.tensor_mul(fac[:], mag[:], rden[:])
                fb = fac[:].unsqueeze(2).broadcast_to(
                    (P, nch, O_SZ, N_NODE))
                nc.vector.tensor_mul(
                    v_out.rearrange("p c (o n) -> p c o n", o=O_SZ),
                    s4, fb)

            def s_banks():
                return [ps_s.tile([128, NO], f32, name=f"s_ps{bc}",
                                  tag=f"s_ps{bc}") for bc in range(2)]

            def s_groups(bank, rhs_sb, glo, ghi):
                """s-matmul groups [glo, ghi) accumulating into bank;
                emitted in pieces so the PE streams behind the mc build."""
                for g in range(glo, ghi):
                    for bc in range(2):
                        nc.tensor.matmul(
                            bank[bc][:],
                            xt8_sb[:, 2 * g:2 * g + 2,
                                   bc * 128:(bc + 1) * 128],
                            rhs_sb[:, 2 * g:2 * g + 2, :],
                            start=(g == 0), stop=(g == NG - 1),
                            perf_mode=PM)

            def s_finish(bank, scale, v8_sb):
                s_sb = wp.tile([128, 2, NO], f32, name="s_sb", tag="s_sb")
                for bc in range(2):
                    nc.scalar.mul(s_sb[:, bc, :], bank[bc][:], scale)
                squash_half(s_sb[:], v8_sb[:], 128, 2, "m", SV,
                            newton_iters=0)
                return s_sb

            def b_update(v8_sb, first, mc_half=None, emit_q=None):
                prb = prb8[:, 0, :].rearrange("p (c n) -> p c n", n=N_NODE)
                ph = [hp.tile([128, NH, NO], bf16, name="ph",
                              tag="ph" + str(h)) for h in range(2)]
                t8s = [hp.tile([128, NH, 80], bf16, name="t8",
                               tag="t8" + str(h)) for h in range(2)]

                def q_run(h, r):
                    qrun = hp.tile([128, 9, NO], bf16, name="qrun",
                                   tag="qr" + str((h * 3 + r) % 3))
                    for gi in range(4):
                        gq = h * 12 + r * 4 + gi
                        q_ps = ps_q.tile([128, 3 * NO], f32,
                                         name="q_ps", tag="q_ps")
                        for s_i in range(3):
                            mch = gq * 3 + s_i
                            nc.tensor.matmul(
                                q_ps[:, s_i * NO:(s_i + 1) * NO],
                                xik8_sb[:, :,
                                        mch * 128:(mch + 1) * 128],
                                v8_sb[:],
                                start=True, stop=True, perf_mode=PM)
                        q3 = q_ps[:].rearrange("p (c f) -> p c f", c=3)
                        if first and h == 0 and r == 0 and gi == 0:
                            dump("d_q1", q3[:, 0, :], NO)
                        lo = (r * 4 + gi) * 3
                        if gi == 3:
                            nc.vector.tensor_mul(
                                ph[h][:, lo:lo + 3, :],
                                wlb_sb[:,
                                       h * NH + lo:h * NH + lo + 3, :],
                                q3)
                        else:
                            nc.scalar.copy(
                                qrun[:, gi * 3:gi * 3 + 3, :], q3)
                    lo = r * 12
                    nc.vector.tensor_mul(
                        ph[h][:, lo:lo + 9, :],
                        wlb_sb[:, h * NH + lo:h * NH + lo + 9, :],
                        qrun[:])
                    if first and h == 0 and r == 0:
                        dump("d_p1", ph[0][:, 0, :], NO)
                    # tree level 1 for this run's 12 chunks, pipelined so
                    # only levels 2-4 remain after the half's last multiply
                    # (GpSimd is too slow here AND its SBUF traffic slows
                    # the DVE ~1.7x — measured; keep the DVE)
                    vh = ph[h][:, lo:lo + 12, :]
                    nc.vector.tensor_add(
                        t8s[h][:, lo:lo + 12, :],
                        vh[:, :, 0:80], vh[:, :, 80:160])

                def finish_half(h):
                    hs = slice(h * NH, (h + 1) * NH)
                    t8 = t8s[h]
                    t4 = hp.tile([128, NH, 40], bf16, name="t4",
                                 tag="t4" + str(h))
                    nc.vector.tensor_add(t4[:], t8[:, :, 0:40],
                                         t8[:, :, 40:80])
                    t2 = hp.tile([128, NH, 20], bf16, name="t2",
                                 tag="t2" + str(h))
                    nc.vector.tensor_add(t2[:], t4[:, :, 0:20],
                                         t4[:, :, 20:40])
                    nc.vector.tensor_add(prb[:, hs, :],
                                         t2[:, :, 0:10], t2[:, :, 10:20])
                    uv_ps = ps_f.tile([128, NH * N_NODE], f32,
                                      name=f"uv_ps{h}", tag=f"uv_ps{h}")
                    W2 = NH * N_NODE
                    nc.tensor.matmul(
                        uv_ps[:], f8_sb[:],
                        prb8[:, :, h * W2:(h + 1) * W2],
                        start=True, stop=True, perf_mode=PM)
                    uv3 = uv_ps[:].rearrange("p (c n) -> p c n", n=N_NODE)
                    if first:
                        nc.scalar.copy(b_sb[:, hs, :], uv3)
                        b_src = uv3
                    else:
                        nc.vector.tensor_add(b_sb[:, hs, :],
                                             b_sb[:, hs, :], uv3)
                        b_src = b_sb[:, hs, :]
                    if first and h == 0:
                        dump("d_pr1", prb[:, 0, :], N_NODE)
                    softmax_part(hs, b_src)
                    if mc_half is not None:
                        mc_half(h)

                # Emit h1's first runs BEFORE h0's tree/uv/softmax block so
                # the in-order PE queue keeps flowing while the DVE tree
                # completes; the next iteration's s-matmul halves stream
                # on the PE behind the finished mc halves.
                for r in range(3):
                    q_run(0, r)
                q_run(1, 0)
                q_run(1, 1)
                finish_half(0)
                q_run(1, 2)
                if emit_q is not None:
                    emit_q(0)
                    emit_q(1)
                finish_half(1)
                if emit_q is not None:
                    emit_q(2)
                    emit_q(3)
                return None

            e_sb = pp.tile([128, NCH, N_NODE], bf16, name="e_sb",
                           tag="e_sb")
            se = pp.tile([128, NCH], f32, name="se", tag="se")
            rse = pp.tile([128, NCH], f32, name="rse", tag="rse")
            rse_x = pp.tile([128, NCH, N_NODE], bf16, name="rse_x",
                            tag="rse_x")
            c_sb = pp.tile([128, NCH, N_NODE], bf16, name="c_sb",
                           tag="c_sb")

            def softmax_part(hs, b_src):
                ncs = hs.stop - hs.start
                nc.scalar.activation(e_sb[:, hs, :], b_src, AF.Exp)
                nc.vector.reduce_sum(se[:, hs], e_sb[:, hs, :], axis=AX.X)
                nc.vector.reciprocal_approx_fast(rse[:, hs], se[:, hs])
                # expand 1/sum to bf16 on the scalar engine so the c
                # multiply packs (2x) without extra DVE work
                nc.scalar.copy(
                    rse_x[:, hs, :],
                    rse[:, hs].unsqueeze(2).broadcast_to(
                        (128, ncs, N_NODE)))
                nc.vector.tensor_mul(c_sb[:, hs, :], e_sb[:, hs, :],
                                     rse_x[:, hs, :])

            def mc_half_fn(mc, wl4_src):
                mc4 = mc[:].rearrange("p c (o n) -> p c o n", o=O_SZ)
                cb = c_sb[:].unsqueeze(2).broadcast_to(
                    (128, NCH, O_SZ, N_NODE))

                def go(h):
                    # slab 0 is the first the next s-matmul consumes ->
                    # fast DVE path; GpSimd (slow but free) covers 1,2
                    # which the PE reaches only ~3.2/4.8us later. h1's
                    # slabs go on the DVE after the half-1 chain.
                    if h == 0:
                        slabs = [(0, nc.vector), (1, nc.gpsimd),
                                 (2, nc.gpsimd)]
                    else:
                        slabs = [(3, nc.vector), (4, nc.vector),
                                 (5, nc.vector)]
                    for sl, eng in slabs:
                        cs = slice(sl * 12, (sl + 1) * 12)
                        eng.tensor_mul(mc4[:, cs], wl4_src[:, cs],
                                       cb[:, cs])
                return go

            # ---------------- iteration 1 (c uniform = 0.1) ----------------
            v8 = wp.tile([128, 2, NO], f8, name="v8", tag="v8")
            bank1 = s_banks()
            s_groups(bank1, wl8_sb, 0, NG)
            s1_sb = s_finish(bank1, 0.1 / SW, v8)
            if _DEBUG:
                dump("d_s1", s1_sb[:].rearrange("p a b -> p (a b)"),
                     2 * NO)
                dump("d_v1", v8[:].rearrange("p a b -> p (a b)"), 2 * NO)
            mc8 = wp.tile([128, NCH, NO], f8, name="mc8", tag="mc8")
            bank2 = s_banks()
            b_update(v8, first=True, mc_half=mc_half_fn(mc8, wl84),
                     emit_q=lambda qi: s_groups(
                         bank2, mc8[:], qi * NG // 4, (qi + 1) * NG // 4))
            if _DEBUG:
                dump("d_b1", b_sb[:].rearrange("p a b -> p (a b)"),
                     NCH * N_NODE)
                dump("d_c1", c_sb[:].rearrange("p a b -> p (a b)"),
                     NCH * N_NODE)

            # ---------------- iteration 2 ----------------
            v8 = wp.tile([128, 2, NO], f8, name="v8b", tag="v8")
            s2_sb = s_finish(bank2, 1.0 / SW, v8)
            if _DEBUG:
                dump("d_s2", s2_sb[:].rearrange("p a b -> p (a b)"),
                     2 * NO)
            mc3 = wp.tile([128, NCH, NO], bf16, name="mc3", tag="mc3")
            s3_ps = ps_s.tile([B_SH, NO], f32, name="s3_ps",
                               tag="s_ps0")

            def s3_groups(clo, chi):
                for c in range(clo, chi):
                    nc.tensor.matmul(s3_ps[:], xts_sb[:, c, :],
                                     mc3[:, c, :],
                                     start=(c == 0), stop=(c == NCH - 1))

            b_update(v8, first=False, mc_half=mc_half_fn(mc3, wlb4),
                     emit_q=lambda qi: s3_groups(
                         qi * NCH // 4, (qi + 1) * NCH // 4))
            if _DEBUG:
                dump("d_b2", b_sb[:].rearrange("p a b -> p (a b)"),
                     NCH * N_NODE)

            # ---------------- iteration 3: bf16, own batch shard ----------
            ssh = wp.tile([B_SH, 1, NO], f32, name="ssh", tag="ssh")
            nc.scalar.copy(ssh[:, 0, :], s3_ps[:])
            if _DEBUG:
                dump("d_s3", ssh[:, 0, :], NO, pdim=B_SH)
            ysh = wp.tile([B_SH, 1, NO], f32, name="ysh", tag="ysh")
            squash_half(ssh[:], ysh[:], B_SH, 1, "s", 1.0, newton_iters=1)
            nc.scalar.dma_start(y_d[0:16, :], ysh[0:16, 0, :])
            nc.sync.dma_start(y_d[16:32, :], ysh[16:32, 0, :])

    nc.compile()
    return nc


def _pack_pm(arr2d, cols):
    """[J, cols] row-major -> [128, NCH*cols] partition-major contiguous."""
    return np.ascontiguousarray(
        arr2d.reshape(NCH, 128, cols).transpose(1, 0, 2).reshape(
            128, NCH * cols))


def _host_prep(x, W):
    """Per-core input dicts; only xts (the 32-col batch shard of x, bf16)
    differs between cores."""
    import ml_dtypes

    bf = ml_dtypes.bfloat16
    f8 = ml_dtypes.float8_e4m3
    x = np.ascontiguousarray(x, dtype=np.float32)
    W = np.ascontiguousarray(W, dtype=np.float32)
    xt = np.ascontiguousarray(x.transpose(2, 1, 0)).reshape(J, B)
    xik = np.ascontiguousarray(x.transpose(0, 2, 1)).reshape(B, J)
    # wl columns in (o, n) order: n innermost
    wl = np.ascontiguousarray(
        (np.float32(0.03) * W[0]).transpose(0, 3, 2, 1)).reshape(J, NO)
    in8 = np.concatenate(
        [(wl * np.float32(SW)).astype(f8), xt.astype(f8)], axis=1)
    in8 = _pack_pm(in8, CW)
    xik8 = np.ascontiguousarray(
        xik.astype(f8).reshape(2, 128, J).transpose(1, 0, 2).reshape(
            128, 2 * J))
    wlb = _pack_pm(wl.astype(bf), NO)
    xtb = xt.astype(bf)
    # F entries 1/(B*SV) = 2^-12: exact in bf16.
    F = (np.kron(np.eye(16, dtype=np.float32),
                 np.ones((8, 8), dtype=np.float32))
         / np.float32(B * SV)).astype(bf)
    base = {"in8": in8, "xik8": xik8, "wlb": wlb, "fmat": F}
    in_maps = []
    for c in range(N_CORES):
        m = dict(base)
        m["xts"] = _pack_pm(np.ascontiguousarray(
            xtb[:, c * B_SH:(c + 1) * B_SH]), B_SH)
        in_maps.append(m)
    return in_maps


def _run(in_maps, trace=False, all_cores=False):
    from concourse.bass_utils import run_bass_kernel_spmd

    if "nc" not in _CACHE:
        _CACHE["nc"] = _build_program()
    nc = _CACHE["nc"]
    kwargs = {}
    if all_cores:
        kwargs["trace_cores"] = list(range(N_CORES))
    res = run_bass_kernel_spmd(nc, in_maps, core_ids=list(range(N_CORES)),
                               trace=trace, **kwargs)
    return res


def kernel(x: np.ndarray, W: np.ndarray) -> np.ndarray:
    in_maps = _host_prep(x, W)
    res = _run(in_maps)
    # y columns are (o, n): reshape and swap back to (n, o)
    v = np.concatenate([res.results[c]["y"] for c in range(N_CORES)], axis=0)
    v = v.reshape(B, O_SZ, N_NODE).transpose(0, 2, 1)
    return np.ascontiguousarray(v).reshape(
        B, N_NODE, O_SZ, 1).astype(np.float32)
